# revision 1
# baseline (speedup 1.0000x reference)
"""Depth-aware 3x3 convolution on 8 Trainium2 NeuronCores (Bass, raw engine blocks).

out[b,o,h,w] = sum_{c,kh,kw} weight[o,c,kh,kw] * x[b,c,h+kh-1,w+kw-1]
                             * exp(-8.3*|depth[b,h,w] - depth[b,h+kh-1,w+kw-1]|)

Sharding: core = 2*b + (h >= 128); each core computes a [32, 128, 256] output
slab from a 130-row padded input frame (1-row halo from the host slice).

Per-core pipeline:
  A. sim: depth rows pixel-major [128, 258]x3 -> |dc-dk| (DVE) -> exp (ACT, bf16)
     -> DRAM simd[9, 32768]
  B. main loop over 16 tiles of 2048 px (8 rows):
     - DMA: x3 chunk [96, 10*258] (3 column-shift blocks stacked on partitions)
     - DMA: broadcast simd rows across 32 partitions -> simrep3 [96, 2048] bf16
     - DVE: xm3 = x3_rows(t) * simrep3  (f32r out)   x3 passes t=0,1,2
     - PE : psum[32, 2048] += w3[:, t].T @ xm3       (K=96, N=512 x4, f32r)
     - ACT: psum -> sbuf f32; DMA out.
"""
import sys

import numpy as np

sys.path.insert(0, "/opt/trn_rl_repo")

import concourse.bass as bass
import concourse.mybir as mybir
from concourse.bass_utils import run_bass_kernel_spmd

F32 = mybir.dt.float32
F32R = mybir.dt.float32r
BF16 = mybir.dt.bfloat16
EXP = mybir.ActivationFunctionType.Exp

B, C, H, W = 4, 32, 256, 256
O = 32
ALPHA = 8.3
R = 128  # output rows per core
WP = W + 2  # padded width
FR = R + 2  # frame rows per core
NPIX = R * W  # 32768
TROWS = 8  # rows per tile
TILE = TROWS * W  # 2048
NT = R // TROWS  # 16
CH_ROWS = TROWS + 2  # x3 chunk rows
MMN = 512  # matmul free-dim chunk
QN = TILE // MMN  # 4


def build_nc():
    nc = bass.Bass("TRN2", target_bir_lowering=False, debug=False, num_devices=8)
    x3_in = nc.declare_dram_parameter("x3", [96, FR * WP], F32, isOutput=False)
    dp_in = nc.declare_dram_parameter("dp", [FR, WP], F32, isOutput=False)
    w3_in = nc.declare_dram_parameter("w3", [96, 96], F32, isOutput=False)
    out_d = nc.declare_dram_parameter("out", [O, NPIX], F32, isOutput=True)
    simd = nc.dram_tensor("simd", [9, NPIX], BF16)
    simd_r = simd.ap().rearrange("k (r w) -> k r w", r=R)

    from contextlib import ExitStack

    ctx = ExitStack()
    with ctx:
        d_sb = ctx.enter_context(nc.sbuf_tensor([128, 3 * WP], F32))
        adiff9 = ctx.enter_context(nc.sbuf_tensor([128, 9 * W], F32))
        sim9 = ctx.enter_context(nc.sbuf_tensor([128, 9 * W], BF16))
        w3_sb = ctx.enter_context(nc.sbuf_tensor([96, 96], F32))
        w3r = ctx.enter_context(nc.sbuf_tensor([96, 96], F32R))
        x3c = ctx.enter_context(nc.sbuf_tensor([96, 2 * CH_ROWS * WP], F32))
        simrep3 = ctx.enter_context(nc.sbuf_tensor([96, 2 * TILE], BF16))
        xm3 = ctx.enter_context(nc.sbuf_tensor([96, 2 * TILE], F32R))
        out_sb = ctx.enter_context(nc.sbuf_tensor([32, 2 * TILE], F32))
        psum = ctx.enter_context(nc.psum_tensor([32, 2 * TILE], F32))
        ld_sem = ctx.enter_context(nc.semaphore("ld_sem"))
        x_e = ctx.enter_context(nc.semaphore("x_e"))
        x_o = ctx.enter_context(nc.semaphore("x_o"))
        sim_dve = ctx.enter_context(nc.semaphore("sim_dve"))
        act_exp = ctx.enter_context(nc.semaphore("act_exp"))
        sim_st = ctx.enter_context(nc.semaphore("sim_st"))
        bc_e = ctx.enter_context(nc.semaphore("bc_e"))
        bc_o = ctx.enter_context(nc.semaphore("bc_o"))
        mod_sem = ctx.enter_context(nc.semaphore("mod_sem"))
        wr_sem = ctx.enter_context(nc.semaphore("wr_sem"))
        pe_sem = ctx.enter_context(nc.semaphore("pe_sem"))
        act_cp = ctx.enter_context(nc.semaphore("act_cp"))
        st_e = ctx.enter_context(nc.semaphore("st_e"))
        st_o = ctx.enter_context(nc.semaphore("st_o"))
        block = ctx.enter_context(nc.Block())

        @block.sync
        def _(sync: bass.BassEngine):
            # startup loads: d (3 row-shifted views), w3
            for t in range(3):
                sync.dma_start(
                    d_sb[:, t * WP : (t + 1) * WP], dp_in[t : t + 128, :]
                ).then_inc(ld_sem, 16)
            sync.dma_start(w3_sb[:], w3_in[:]).then_inc(ld_sem, 16)
            # sim -> DRAM
            for k in range(9):
                sync.wait_ge(act_exp, k + 1)
                sync.dma_start(
                    simd_r[k], sim9[:, k * W : (k + 1) * W]
                ).then_inc(sim_st, 16)
            # main loop
            for i in range(NT):
                bi = i % 2
                # x3 chunk for tile i
                if i >= 2:
                    sync.wait_ge(mod_sem, 3 * (i - 2) + 3)
                sync.dma_start(
                    x3c[:, bi * CH_ROWS * WP : (bi + 1) * CH_ROWS * WP],
                    x3_in[:, i * TROWS * WP : (i * TROWS + CH_ROWS) * WP],
                ).then_inc(x_e if bi == 0 else x_o, 16)
                # broadcast sim rows for the 3 passes
                if i == 0:
                    sync.wait_ge(sim_st, 9 * 16)
                for t in range(3):
                    s = 3 * i + t
                    sb = s % 2
                    if s >= 2:
                        sync.wait_ge(mod_sem, s - 1)
                    for j in range(3):
                        sync.dma_start(
                            simrep3[
                                32 * j : 32 * (j + 1),
                                sb * TILE : (sb + 1) * TILE,
                            ],
                            simd[
                                3 * t + j : 3 * t + j + 1,
                                i * TILE : (i + 1) * TILE,
                            ].to_broadcast((32, TILE)),
                        ).then_inc(bc_e if sb == 0 else bc_o, 16)
                # store tile i-1
                if i >= 1:
                    sync.wait_ge(act_cp, i)
                    sync.dma_start(
                        out_d[:, (i - 1) * TILE : i * TILE],
                        out_sb[:, ((i - 1) % 2) * TILE : ((i - 1) % 2 + 1) * TILE],
                    ).then_inc(st_e if (i - 1) % 2 == 0 else st_o, 16)
            sync.wait_ge(act_cp, NT)
            sync.dma_start(
                out_d[:, (NT - 1) * TILE :],
                out_sb[:, ((NT - 1) % 2) * TILE : ((NT - 1) % 2 + 1) * TILE],
            ).then_inc(st_e if (NT - 1) % 2 == 0 else st_o, 16)

        @block.vector
        def _(vector):
            # sim phase: diff + abs per tap
            vector.wait_ge(ld_sem, 64)
            for t in range(3):
                for j in range(3):
                    k = 3 * t + j
                    vector.tensor_sub(
                        adiff9[:, k * W : (k + 1) * W],
                        d_sb[:, WP + 1 : WP + 1 + W],
                        d_sb[:, t * WP + j : t * WP + j + W],
                    )
                    vector.drain()
                    vector.scalar_tensor_tensor(
                        adiff9[:, k * W : (k + 1) * W],
                        adiff9[:, k * W : (k + 1) * W],
                        -1.0,
                        adiff9[:, k * W : (k + 1) * W],
                        op0=mybir.AluOpType.mult,
                        op1=mybir.AluOpType.max,
                    ).then_inc(sim_dve, 1)
            # round weights to f32r
            vector.wait_ge(ld_sem, 64)
            vector.tensor_copy(w3r[:], w3_sb[:]).then_inc(wr_sem, 1)
            # modulation loop
            for i in range(NT):
                bi = i % 2
                vector.wait_ge(x_e if bi == 0 else x_o, 16 * (i // 2 + 1))
                for t in range(3):
                    s = 3 * i + t
                    sb = s % 2
                    vector.wait_ge(bc_e if sb == 0 else bc_o, 48 * (s // 2 + 1))
                    if s >= 2:
                        vector.wait_ge(pe_sem, s - 1)
                    xv = x3c[:, bi * CH_ROWS * WP : (bi + 1) * CH_ROWS * WP]
                    xv = xv.rearrange("p (r w) -> p r w", w=WP)
                    vector.tensor_mul(
                        xm3[:, sb * TILE : (sb + 1) * TILE].rearrange(
                            "p (r w) -> p r w", w=W
                        ),
                        xv[:, t : t + TROWS, 1 : 1 + W],
                        simrep3[:, sb * TILE : (sb + 1) * TILE].rearrange(
                            "p (r w) -> p r w", w=W
                        ),
                    ).then_inc(mod_sem, 1)

        @block.tensor
        def _(tensor):
            tensor.wait_ge(wr_sem, 1)
            for i in range(NT):
                bi = i % 2
                if i >= 2:
                    tensor.wait_ge(act_cp, i - 1)
                for t in range(3):
                    s = 3 * i + t
                    sb = s % 2
                    tensor.wait_ge(mod_sem, s + 1)
                    for q in range(QN):
                        mm = tensor.matmul(
                            psum[:, bi * TILE + q * MMN : bi * TILE + (q + 1) * MMN],
                            w3r[:, 32 * t : 32 * (t + 1)],
                            xm3[:, sb * TILE + q * MMN : sb * TILE + (q + 1) * MMN],
                            start=(t == 0),
                            stop=(t == 2),
                        )
                        if q == QN - 1:
                            mm.then_inc(pe_sem, 1)

        @block.scalar
        def _(scalar):
            # exp per tap (bf16 out)
            for k in range(9):
                scalar.wait_ge(sim_dve, k + 1)
                scalar.activation(
                    sim9[:, k * W : (k + 1) * W],
                    adiff9[:, k * W : (k + 1) * W],
                    EXP,
                    scale=-ALPHA,
                ).then_inc(act_exp, 1)
            # psum -> sbuf copies
            for i in range(NT):
                bi = i % 2
                scalar.wait_ge(pe_sem, 3 * i + 3)
                if i >= 2:
                    scalar.wait_ge(st_e if i % 2 == 0 else st_o, 16 * (i // 2))
                scalar.copy(
                    out_sb[:, bi * TILE : (bi + 1) * TILE],
                    psum[:, bi * TILE : (bi + 1) * TILE],
                ).then_inc(act_cp, 1)

    return nc


_NC_CACHE = None


def _get_nc():
    global _NC_CACHE
    if _NC_CACHE is None:
        _NC_CACHE = build_nc()
    return _NC_CACHE


def _prep_core(x, depth, weight, core):
    b, half = core // 2, core % 2
    r0 = half * R
    # padded frame [C, FR, WP]: image rows r0-1 .. r0+R, zero-padded
    xpad = np.zeros((C, FR, WP), dtype=np.float32)
    dpad = np.zeros((FR, WP), dtype=np.float32)
    lo, hi = r0 - 1, r0 + R + 1
    slo, shi = max(lo, 0), min(hi, H)
    xpad[:, slo - lo : shi - lo, 1 : 1 + W] = x[b, :, slo:shi, :]
    dpad[slo - lo : shi - lo, 1 : 1 + W] = depth[b, 0, slo:shi, :]
    # x3: 3 column-shift blocks stacked on partitions
    x3 = np.zeros((3, C, FR, WP), dtype=np.float32)
    x3[0, :, :, 1:] = xpad[:, :, :-1]  # j=0: w-1
    x3[1] = xpad  # j=1: w
    x3[2, :, :, :-1] = xpad[:, :, 1:]  # j=2: w+1
    return {
        "x3": x3.reshape(3 * C, FR * WP),
        "dp": dpad,
        "w3": None,  # filled by caller (shared)
    }


def kernel(x, depth, weight):
    x = np.ascontiguousarray(x, dtype=np.float32)
    depth = np.ascontiguousarray(depth, dtype=np.float32)
    weight = np.ascontiguousarray(weight, dtype=np.float32)

    # w3[32j + c, 32t + o] = weight[o, c, t, j]
    w3 = np.transpose(weight, (3, 1, 2, 0)).reshape(96, 96).copy()

    in_maps = []
    for core in range(8):
        m = _prep_core(x, depth, weight, core)
        m["w3"] = w3
        in_maps.append(m)

    nc = _get_nc()
    res = run_bass_kernel_spmd(nc, in_maps, list(range(8)))

    out = np.empty((B, O, H, W), dtype=np.float32)
    for core in range(8):
        b, half = core // 2, core % 2
        out[b, :, half * R : (half + 1) * R, :] = res.results[core]["out"].reshape(
            O, R, W
        )
    return out



# revision 2
# speedup vs baseline: 1.0011x; 1.0011x over previous
"""Depth-aware 3x3 convolution on 8 Trainium2 NeuronCores (Bass, raw engine blocks).

out[b,o,h,w] = sum_{c,kh,kw} weight[o,c,kh,kw] * x[b,c,h+kh-1,w+kw-1]
                             * exp(-8.3*|depth[b,h,w] - depth[b,h+kh-1,w+kw-1]|)

Sharding: core = 2*b + (h >= 128); each core computes a [32, 128, 256] output
slab from a 130-row padded input frame (1-row halo from the host slice).

v2 pipeline (bf16 datapath):
  A. sim: depth rows pixel-major [128, 258]x3 -> |dc-dk| (DVE) -> exp (ACT,
     bf16) -> one DMA to DRAM simd[9, 32768].
  B. main loop over 8 tiles of 4096 px (16 rows):
     - 3 DMAs: x chunk [32, 18*258] bf16 loaded at flat offsets j-1 into the
       three partition groups of x3c [96, 18*258] (column shifts for free).
     - 3 DMAs: broadcast simd rows {j,3+j,6+j} across 32 partitions each ->
       simrep [96, 3*4096] bf16 (t-major free layout).
     - DVE: xm3 = x3c rows(t) * simrep(t)  (bf16, 2x mode)   passes t=0,1,2
     - PE : psum[32@g, 4096] += w3[:, t].T @ xm3   (K=96, N=512 x8, bf16)
       with psum double-buffered across partition groups g = tile%2.
     - ACT: psum -> out_sb bf16; DMA out.
"""
import sys

import numpy as np

sys.path.insert(0, "/opt/trn_rl_repo")

import concourse.bass as bass
import concourse.mybir as mybir
from concourse.bass_utils import run_bass_kernel_spmd

F32 = mybir.dt.float32
BF16 = mybir.dt.bfloat16
EXP = mybir.ActivationFunctionType.Exp
COPY = mybir.ActivationFunctionType.Copy

B, C, H, W = 4, 32, 256, 256
O = 32
ALPHA = 8.3
R = 128  # output rows per core
WP = W + 2  # padded width
FR = R + 2  # frame rows per core
NPIX = R * W  # 32768
XLEN = FR * WP + 2  # flat x frame + 1-elem guard pads on both ends
TROWS = 16  # rows per tile
TILE = TROWS * W  # 4096
NT = R // TROWS  # 8
CH_ROWS = TROWS + 2  # x chunk rows
CH_FREE = CH_ROWS * WP  # 4644
MMN = 512  # matmul free-dim chunk (one PSUM bank)
QN = TILE // MMN  # 8


def build_nc():
    nc = bass.Bass("TRN2", target_bir_lowering=False, debug=False, num_devices=8)
    xb_in = nc.declare_dram_parameter("xb", [C, XLEN], BF16, isOutput=False)
    dp_in = nc.declare_dram_parameter("dp", [FR, WP], F32, isOutput=False)
    w3_in = nc.declare_dram_parameter("w3", [96, 96], BF16, isOutput=False)
    out_d = nc.declare_dram_parameter("outd", [O, NPIX], BF16, isOutput=True)
    simd = nc.dram_tensor("simd", [9, NPIX], BF16)

    from contextlib import ExitStack

    ctx = ExitStack()
    with ctx:
        d_sb = ctx.enter_context(nc.sbuf_tensor([128, 3 * WP], F32))
        adiff9 = ctx.enter_context(nc.sbuf_tensor([128, 9 * W], F32))
        sim9 = ctx.enter_context(nc.sbuf_tensor([128, 9 * W], BF16))
        w3_sb = ctx.enter_context(nc.sbuf_tensor([96, 96], BF16))
        x3c = ctx.enter_context(nc.sbuf_tensor([96, 2 * CH_FREE], BF16))
        simrep = ctx.enter_context(nc.sbuf_tensor([96, 2 * 3 * TILE], BF16))
        xm3 = ctx.enter_context(nc.sbuf_tensor([96, 2 * TILE], BF16))
        out_sb = ctx.enter_context(nc.sbuf_tensor([32, 2 * TILE], BF16))
        psum = ctx.enter_context(nc.psum_tensor([64, TILE], F32))
        ld_d = ctx.enter_context(nc.semaphore("ld_d"))
        ld_w = ctx.enter_context(nc.semaphore("ld_w"))
        x_e = ctx.enter_context(nc.semaphore("x_e"))
        x_o = ctx.enter_context(nc.semaphore("x_o"))
        sim_dve = ctx.enter_context(nc.semaphore("sim_dve"))
        act_exp = ctx.enter_context(nc.semaphore("act_exp"))
        sim_st = ctx.enter_context(nc.semaphore("sim_st"))
        bc_e = ctx.enter_context(nc.semaphore("bc_e"))
        bc_o = ctx.enter_context(nc.semaphore("bc_o"))
        mod_sem = ctx.enter_context(nc.semaphore("mod_sem"))
        pe_sem = ctx.enter_context(nc.semaphore("pe_sem"))
        act_cp = ctx.enter_context(nc.semaphore("act_cp"))
        st_e = ctx.enter_context(nc.semaphore("st_e"))
        st_o = ctx.enter_context(nc.semaphore("st_o"))
        block = ctx.enter_context(nc.Block())

        @block.sync
        def _(sync: bass.BassEngine):
            # startup loads: d (3 row-shifted views), w3
            for t in range(3):
                sync.dma_start(
                    d_sb[:, t * WP : (t + 1) * WP], dp_in[t : t + 128, :]
                ).then_inc(ld_d, 16)
            sync.dma_start(w3_sb[:], w3_in[:]).then_inc(ld_w, 16)
            # x chunks for tiles 0,1 (no deps; keep SP queue moving while
            # the sim phase runs)
            for i in range(2):
                for j in range(3):
                    off = 16 * i * WP + j
                    sync.dma_start(
                        x3c[
                            32 * j : 32 * (j + 1),
                            (i % 2) * CH_FREE : (i % 2 + 1) * CH_FREE,
                        ],
                        xb_in[:, off : off + CH_FREE],
                    ).then_inc(x_e if i % 2 == 0 else x_o, 16)
            # sim -> DRAM (single DMA, 512B rows)
            sync.wait_ge(act_exp, 9)
            sync.dma_start(
                simd.ap().rearrange("k (r w) -> r k w", r=128),
                sim9[:].rearrange("p (k w) -> p k w", k=9),
            ).then_inc(sim_st, 16)
            # main loop
            for i in range(NT):
                bi = i % 2
                if i >= 2:
                    # x3c/simrep buf bi free once tile i-2's 3 muls are done
                    sync.wait_ge(mod_sem, 3 * (i - 2) + 3)
                    for j in range(3):
                        off = 16 * i * WP + j
                        sync.dma_start(
                            x3c[
                                32 * j : 32 * (j + 1),
                                bi * CH_FREE : (bi + 1) * CH_FREE,
                            ],
                            xb_in[:, off : off + CH_FREE],
                        ).then_inc(x_e if bi == 0 else x_o, 16)
                if i == 0:
                    sync.wait_ge(sim_st, 16)
                for j in range(3):
                    src = (
                        simd.ap()[:, i * TILE : (i + 1) * TILE]
                        .rearrange("(t j) n -> j t n", j=3)[j]
                        .unsqueeze(0)
                        .broadcast_to((32, 3, TILE))
                    )
                    sync.dma_start(
                        simrep[
                            32 * j : 32 * (j + 1),
                            bi * 3 * TILE : (bi + 1) * 3 * TILE,
                        ].rearrange("p (t n) -> p t n", t=3),
                        src,
                    ).then_inc(bc_e if bi == 0 else bc_o, 16)
                # store tile i-1
                if i >= 1:
                    sync.wait_ge(act_cp, i)
                    sync.dma_start(
                        out_d[:, (i - 1) * TILE : i * TILE],
                        out_sb[:, ((i - 1) % 2) * TILE : ((i - 1) % 2 + 1) * TILE],
                    ).then_inc(st_e if (i - 1) % 2 == 0 else st_o, 16)
            sync.wait_ge(act_cp, NT)
            sync.dma_start(
                out_d[:, (NT - 1) * TILE :],
                out_sb[:, ((NT - 1) % 2) * TILE : ((NT - 1) % 2 + 1) * TILE],
            ).then_inc(st_e if (NT - 1) % 2 == 0 else st_o, 16)

        @block.vector
        def _(vector):
            # sim phase: diff + abs per tap
            vector.wait_ge(ld_d, 48)
            for t in range(3):
                for j in range(3):
                    k = 3 * t + j
                    vector.tensor_sub(
                        adiff9[:, k * W : (k + 1) * W],
                        d_sb[:, WP + 1 : WP + 1 + W],
                        d_sb[:, t * WP + j : t * WP + j + W],
                    )
                    vector.drain()
                    vector.scalar_tensor_tensor(
                        adiff9[:, k * W : (k + 1) * W],
                        adiff9[:, k * W : (k + 1) * W],
                        -1.0,
                        adiff9[:, k * W : (k + 1) * W],
                        op0=mybir.AluOpType.mult,
                        op1=mybir.AluOpType.max,
                    ).then_inc(sim_dve, 1)
            # modulation loop
            for i in range(NT):
                bi = i % 2
                for t in range(3):
                    s = 3 * i + t
                    sb = s % 2
                    if t == 0:
                        vector.wait_ge(x_e if bi == 0 else x_o, 48 * (i // 2 + 1))
                        vector.wait_ge(bc_e if bi == 0 else bc_o, 48 * (i // 2 + 1))
                    if s >= 2:
                        vector.wait_ge(pe_sem, s - 1)
                    xv = x3c[:, bi * CH_FREE : (bi + 1) * CH_FREE].rearrange(
                        "p (r w) -> p r w", w=WP
                    )
                    sv = simrep[
                        :, bi * 3 * TILE + t * TILE : bi * 3 * TILE + (t + 1) * TILE
                    ].rearrange("p (r w) -> p r w", w=W)
                    vector.tensor_mul(
                        xm3[:, sb * TILE : (sb + 1) * TILE].rearrange(
                            "p (r w) -> p r w", w=W
                        ),
                        xv[:, t : t + TROWS, 1 : 1 + W],
                        sv,
                    ).then_inc(mod_sem, 1)

        @block.tensor
        def _(tensor):
            tensor.wait_ge(ld_w, 16)
            for i in range(NT):
                g = i % 2
                if i >= 2:
                    tensor.wait_ge(act_cp, i - 1)
                for t in range(3):
                    s = 3 * i + t
                    sb = s % 2
                    tensor.wait_ge(mod_sem, s + 1)
                    for q in range(QN):
                        mm = tensor.matmul(
                            psum[
                                32 * g : 32 * (g + 1), q * MMN : (q + 1) * MMN
                            ],
                            w3_sb[:, 32 * t : 32 * (t + 1)],
                            xm3[:, sb * TILE + q * MMN : sb * TILE + (q + 1) * MMN],
                            start=(t == 0),
                            stop=(t == 2),
                        )
                        if q == QN - 1:
                            mm.then_inc(pe_sem, 1)

        @block.scalar
        def _(scalar):
            # exp per tap (bf16 out)
            for k in range(9):
                scalar.wait_ge(sim_dve, k + 1)
                scalar.activation(
                    sim9[:, k * W : (k + 1) * W],
                    adiff9[:, k * W : (k + 1) * W],
                    EXP,
                    scale=-ALPHA,
                ).then_inc(act_exp, 1)
            # psum -> sbuf copies (f32 -> bf16)
            for i in range(NT):
                bi = i % 2
                g = i % 2
                scalar.wait_ge(pe_sem, 3 * i + 3)
                if i >= 2:
                    scalar.wait_ge(st_e if bi == 0 else st_o, 16 * (i // 2))
                scalar.activation(
                    out_sb[:, bi * TILE : (bi + 1) * TILE],
                    psum[32 * g : 32 * (g + 1), :],
                    COPY,
                ).then_inc(act_cp, 1)

    return nc


_NC_CACHE = None


def _get_nc():
    global _NC_CACHE
    if _NC_CACHE is None:
        _NC_CACHE = build_nc()
    return _NC_CACHE


def _prep_core(x_bf, depth, core):
    import ml_dtypes

    b, half = core // 2, core % 2
    r0 = half * R
    # padded frame rows r0-1 .. r0+R (inclusive), zero-padded cols
    xpad = np.zeros((C, FR, WP), dtype=ml_dtypes.bfloat16)
    dpad = np.zeros((FR, WP), dtype=np.float32)
    lo, hi = r0 - 1, r0 + R + 1
    slo, shi = max(lo, 0), min(hi, H)
    xpad[:, slo - lo : shi - lo, 1 : 1 + W] = x_bf[b, :, slo:shi, :]
    dpad[slo - lo : shi - lo, 1 : 1 + W] = depth[b, 0, slo:shi, :]
    xb = np.zeros((C, XLEN), dtype=ml_dtypes.bfloat16)
    xb[:, 1 : 1 + FR * WP] = xpad.reshape(C, FR * WP)
    return {"xb": xb, "dp": dpad, "w3": None}


def kernel(x, depth, weight):
    import ml_dtypes

    x = np.ascontiguousarray(x, dtype=np.float32)
    depth = np.ascontiguousarray(depth, dtype=np.float32)
    weight = np.ascontiguousarray(weight, dtype=np.float32)

    x_bf = x.astype(ml_dtypes.bfloat16)
    # w3[32j + c, 32t + o] = weight[o, c, t, j]
    w3 = (
        np.transpose(weight, (3, 1, 2, 0))
        .reshape(96, 96)
        .astype(ml_dtypes.bfloat16)
    )

    in_maps = []
    for core in range(8):
        m = _prep_core(x_bf, depth, core)
        m["w3"] = w3
        in_maps.append(m)

    nc = _get_nc()
    res = run_bass_kernel_spmd(nc, in_maps, list(range(8)))

    out = np.empty((B, O, H, W), dtype=np.float32)
    for core in range(8):
        b, half = core // 2, core % 2
        out[b, :, half * R : (half + 1) * R, :] = (
            res.results[core]["outd"].astype(np.float32).reshape(O, R, W)
        )
    return out


# revision 6
# speedup vs baseline: 3.2484x; 3.2447x over previous
"""Depth-aware 3x3 convolution on 8 Trainium2 NeuronCores (Bass, raw engine blocks).

out[b,o,h,w] = sum_{c,kh,kw} weight[o,c,kh,kw] * x[b,c,h+kh-1,w+kw-1]
                             * exp(-8.3*|depth[b,h,w] - depth[b,h+kh-1,w+kw-1]|)

Sharding: core = 2*b + (h >= 128); each core computes a [32, 128, 256] output
slab from a 130-row padded input frame (1-row halo from the host slice).

v3 pipeline (bf16 datapath, DMA spread across SP + GpSimd + ACT queues):
  A. sim: one DMA loads 3 row-shifted depth views; merged DVE sub+abs and one
     ACT exp produce sim9 bf16; one DMA stores simd[9, 32768] to DRAM.
  B. main loop over 8 tiles of 4096 px (16 rows):
     - 1 DMA: x chunk loaded 3x at flat offsets j-1 into the three partition
       groups of x3c [96, 18*258] bf16 (column shifts via a [1,3] lead dim).
     - 1 DMA: broadcast simd rows across 32 partitions per j group ->
       simrep [96, 3*4096] bf16 (t-major free layout).
     - DVE: xm3[:, t] = x3c rows(t..t+16) * simrep(t)  (bf16 2x)  t=0,1,2
     - PE : psum[32@g, 4096] += w3[:, t].T @ xm3[:, t]  (K=96, N=512 x8, bf16)
       psum double-buffered across partition groups g = tile%2.
     - ACT: psum -> out_sb bf16, then ACT-issued DMA out.
  x3c/brc DMAs alternate between the SP and GpSimd queues per tile.
"""
import sys

import numpy as np

sys.path.insert(0, "/opt/trn_rl_repo")

import concourse.bass as bass
import concourse.mybir as mybir
from concourse.bass_utils import run_bass_kernel_spmd

F32 = mybir.dt.float32
BF16 = mybir.dt.bfloat16
EXP = mybir.ActivationFunctionType.Exp
COPY = mybir.ActivationFunctionType.Copy

B, C, H, W = 4, 32, 256, 256
O = 32
ALPHA = 8.3
R = 128  # output rows per core
WP = W + 2  # padded width
FR = R + 2  # frame rows per core
NPIX = R * W  # 32768
XLEN = FR * WP + 2  # flat x frame + 1-elem guard pads on both ends
TROWS = 16  # rows per tile
TILE = TROWS * W  # 4096
NT = R // TROWS  # 8
CH_ROWS = TROWS + 2  # x chunk rows
CH_FREE = CH_ROWS * WP  # 4644
MMN = 512  # matmul free-dim chunk (one PSUM bank)
QN = TILE // MMN  # 8
T3 = 3 * TILE


def build_nc():
    nc = bass.Bass("TRN2", target_bir_lowering=False, debug=False, num_devices=8)
    xb_in = nc.declare_dram_parameter("xb", [C, XLEN], BF16, isOutput=False)
    dp_in = nc.declare_dram_parameter("dp", [FR, WP], F32, isOutput=False)
    w3_in = nc.declare_dram_parameter("w3", [96, 96], BF16, isOutput=False)
    out_d = nc.declare_dram_parameter("outd", [O, NPIX], BF16, isOutput=True)
    simd = nc.dram_tensor("simd", [9, NPIX], BF16)

    from contextlib import ExitStack

    ctx = ExitStack()
    with ctx:
        d_sb = ctx.enter_context(nc.sbuf_tensor([128, 3 * WP], F32))
        adiff9 = ctx.enter_context(nc.sbuf_tensor([128, 9 * W], F32))
        sim9 = ctx.enter_context(nc.sbuf_tensor([128, 9 * W], BF16))
        w3_sb = ctx.enter_context(nc.sbuf_tensor([96, 96], BF16))
        x3c = ctx.enter_context(nc.sbuf_tensor([96, 2 * CH_FREE], BF16))
        simrep = ctx.enter_context(nc.sbuf_tensor([96, 2 * T3], BF16))
        xm3 = ctx.enter_context(nc.sbuf_tensor([96, 2 * T3], BF16))
        out_sb = ctx.enter_context(nc.sbuf_tensor([32, 2 * TILE], BF16))
        psum = ctx.enter_context(nc.psum_tensor([64, TILE], F32))
        ld_d = ctx.enter_context(nc.semaphore("ld_d"))
        ld_w = ctx.enter_context(nc.semaphore("ld_w"))
        x_e = ctx.enter_context(nc.semaphore("x_e"))
        x_o = ctx.enter_context(nc.semaphore("x_o"))
        sim_dve = ctx.enter_context(nc.semaphore("sim_dve"))
        act_exp = ctx.enter_context(nc.semaphore("act_exp"))
        sim_st = ctx.enter_context(nc.semaphore("sim_st"))
        bc_e = ctx.enter_context(nc.semaphore("bc_e"))
        bc_o = ctx.enter_context(nc.semaphore("bc_o"))
        mod_sem = ctx.enter_context(nc.semaphore("mod_sem"))
        pe_sem = ctx.enter_context(nc.semaphore("pe_sem"))
        act_cp = ctx.enter_context(nc.semaphore("act_cp"))
        st_e = ctx.enter_context(nc.semaphore("st_e"))
        st_o = ctx.enter_context(nc.semaphore("st_o"))
        block = ctx.enter_context(nc.Block())

        def rap(base_ap, offset, dims):
            return bass.AP(tensor=base_ap.tensor, offset=offset, ap=dims)

        def x3c_load(eng, i, sem):
            # one DMA: 3 column shifts x 32 channels on partitions
            bi = i % 2
            src = rap(
                xb_in.ap(),
                16 * i * WP,
                [[1, 3], [XLEN, C], [1, CH_FREE]],
            )
            eng.dma_start(
                x3c[:, bi * CH_FREE : (bi + 1) * CH_FREE], src
            ).then_inc(sem, 16)

        def brc_load(eng, i, sem):
            # 3 DMAs (one per t): simd rows {3t, 3t+1, 3t+2} tile window
            # replicated to 32 partitions per j group
            bi = i % 2
            for t in range(3):
                src = rap(
                    simd.ap(),
                    3 * t * NPIX + i * TILE,
                    [[NPIX, 3], [0, C], [1, TILE]],
                )
                eng.dma_start(
                    simrep[:, bi * T3 + t * TILE : bi * T3 + (t + 1) * TILE],
                    src,
                ).then_inc(sem, 16)

        @block.sync
        def _(sync: bass.BassEngine):
            # one DMA: 3 row-shifted depth views [128, 3, 258]
            dsrc = rap(dp_in.ap(), 0, [[WP, 128], [WP, 3], [1, WP]])
            sync.dma_start(
                d_sb[:].rearrange("p (t w) -> p t w", t=3), dsrc
            ).then_inc(ld_d, 16)
            sync.dma_start(w3_sb[:], w3_in[:]).then_inc(ld_w, 16)
            x3c_load(sync, 0, x_e)
            # sim -> DRAM (single DMA, 512B rows)
            sync.wait_ge(act_exp, 1)
            sync.dma_start(
                simd.ap().rearrange("k (r w) -> r k w", r=128),
                sim9[:].rearrange("p (k w) -> p k w", k=9),
            ).then_inc(sim_st, 16)
            # odd-tile broadcasts, even-tile (>=2) x loads
            for i in range(NT):
                if i % 2 == 1:
                    sync.wait_ge(sim_st, 16)
                    if i >= 2:
                        sync.wait_ge(mod_sem, i - 1)
                    brc_load(sync, i, bc_o)
                elif i >= 2:
                    sync.wait_ge(mod_sem, i - 1)
                    x3c_load(sync, i, x_e)

        @block.gpsimd
        def _(pool):
            x3c_load(pool, 1, x_o)
            for i in range(NT):
                if i % 2 == 0:
                    pool.wait_ge(sim_st, 16)
                    if i >= 2:
                        pool.wait_ge(mod_sem, i - 1)
                    brc_load(pool, i, bc_e)
                elif i >= 3:
                    pool.wait_ge(mod_sem, i - 1)
                    x3c_load(pool, i, x_o)

        @block.vector
        def _(vector):
            # sim phase: merged diff + abs over all 9 taps
            vector.wait_ge(ld_d, 16)
            d_ap = d_sb[:, 0:1]
            ad_ap = adiff9[:, 0:1]
            center = rap(d_ap, WP + 1, [[3 * WP, 128], [0, 3], [0, 3], [1, W]])
            wins = rap(d_ap, 0, [[3 * WP, 128], [WP, 3], [1, 3], [1, W]])
            adiff_v = rap(ad_ap, 0, [[9 * W, 128], [3 * W, 3], [W, 3], [1, W]])
            vector.tensor_sub(adiff_v, center, wins)
            vector.drain()
            vector.scalar_tensor_tensor(
                adiff_v,
                adiff_v,
                -1.0,
                adiff_v,
                op0=mybir.AluOpType.mult,
                op1=mybir.AluOpType.max,
            ).then_inc(sim_dve, 1)
            # modulation loop
            for i in range(NT):
                bi = i % 2
                vector.wait_ge(x_e if bi == 0 else x_o, 16 * (i // 2 + 1))
                vector.wait_ge(bc_e if bi == 0 else bc_o, 48 * (i // 2 + 1))
                if i >= 2:
                    vector.wait_ge(pe_sem, i - 1)
                xv = x3c[:, bi * CH_FREE : (bi + 1) * CH_FREE].rearrange(
                    "p (r w) -> p r w", w=WP
                )
                for t in range(3):
                    sv = simrep[
                        :, bi * T3 + t * TILE : bi * T3 + (t + 1) * TILE
                    ].rearrange("p (r w) -> p r w", w=W)
                    mm = vector.tensor_mul(
                        xm3[
                            :, bi * T3 + t * TILE : bi * T3 + (t + 1) * TILE
                        ].rearrange("p (r w) -> p r w", w=W),
                        xv[:, t : t + TROWS, 1 : 1 + W],
                        sv,
                    )
                    if t == 2:
                        mm.then_inc(mod_sem, 1)

        @block.tensor
        def _(tensor):
            tensor.wait_ge(ld_w, 16)
            for i in range(NT):
                bi = i % 2
                g = i % 2
                tensor.wait_ge(mod_sem, i + 1)
                if i >= 2:
                    tensor.wait_ge(act_cp, i - 1)
                for t in range(3):
                    for q in range(QN):
                        mm = tensor.matmul(
                            psum[32 * g : 32 * (g + 1), q * MMN : (q + 1) * MMN],
                            w3_sb[:, 32 * t : 32 * (t + 1)],
                            xm3[
                                :,
                                bi * T3
                                + t * TILE
                                + q * MMN : bi * T3
                                + t * TILE
                                + (q + 1) * MMN,
                            ],
                            start=(t == 0),
                            stop=(t == 2),
                        )
                        if t == 2 and q == QN - 1:
                            mm.then_inc(pe_sem, 1)

        @block.scalar
        def _(scalar):
            # merged exp over all 9 taps (bf16 out)
            scalar.wait_ge(sim_dve, 1)
            scalar.activation(
                sim9[:], adiff9[:], EXP, scale=-ALPHA
            ).then_inc(act_exp, 1)
            # psum -> sbuf copies (f32 -> bf16) + ACT-issued output stores
            for i in range(NT):
                bi = i % 2
                g = i % 2
                scalar.wait_ge(pe_sem, i + 1)
                if i >= 2:
                    scalar.wait_ge(st_e if bi == 0 else st_o, 16 * (i // 2))
                scalar.activation(
                    out_sb[:, bi * TILE : (bi + 1) * TILE],
                    psum[32 * g : 32 * (g + 1), :],
                    COPY,
                ).then_inc(act_cp, 1)
                scalar.wait_ge(act_cp, i + 1)
                scalar.dma_start(
                    out_d[:, i * TILE : (i + 1) * TILE],
                    out_sb[:, bi * TILE : (bi + 1) * TILE],
                ).then_inc(st_e if bi == 0 else st_o, 16)

    return nc


_NC_CACHE = None


def _get_nc():
    global _NC_CACHE
    if _NC_CACHE is None:
        _NC_CACHE = build_nc()
    return _NC_CACHE


def _prep_core(x_bf, depth, core):
    import ml_dtypes

    b, half = core // 2, core % 2
    r0 = half * R
    # padded frame rows r0-1 .. r0+R (inclusive), zero-padded cols
    xpad = np.zeros((C, FR, WP), dtype=ml_dtypes.bfloat16)
    dpad = np.zeros((FR, WP), dtype=np.float32)
    lo, hi = r0 - 1, r0 + R + 1
    slo, shi = max(lo, 0), min(hi, H)
    xpad[:, slo - lo : shi - lo, 1 : 1 + W] = x_bf[b, :, slo:shi, :]
    dpad[slo - lo : shi - lo, 1 : 1 + W] = depth[b, 0, slo:shi, :]
    xb = np.zeros((C, XLEN), dtype=ml_dtypes.bfloat16)
    xb[:, 1 : 1 + FR * WP] = xpad.reshape(C, FR * WP)
    return {"xb": xb, "dp": dpad, "w3": None}


def kernel(x, depth, weight):
    import ml_dtypes

    x = np.ascontiguousarray(x, dtype=np.float32)
    depth = np.ascontiguousarray(depth, dtype=np.float32)
    weight = np.ascontiguousarray(weight, dtype=np.float32)

    x_bf = x.astype(ml_dtypes.bfloat16)
    # w3[32j + c, 32t + o] = weight[o, c, t, j]
    w3 = (
        np.transpose(weight, (3, 1, 2, 0))
        .reshape(96, 96)
        .astype(ml_dtypes.bfloat16)
    )

    in_maps = []
    for core in range(8):
        m = _prep_core(x_bf, depth, core)
        m["w3"] = w3
        in_maps.append(m)

    nc = _get_nc()
    res = run_bass_kernel_spmd(nc, in_maps, list(range(8)))

    out = np.empty((B, O, H, W), dtype=np.float32)
    for core in range(8):
        b, half = core // 2, core % 2
        out[b, :, half * R : (half + 1) * R, :] = (
            res.results[core]["outd"].astype(np.float32).reshape(O, R, W)
        )
    return out


# revision 17
# speedup vs baseline: 4.4821x; 1.3798x over previous
"""Depth-aware 3x3 convolution on 8 Trainium2 NeuronCores (Bass, raw engine blocks).

out[b,o,h,w] = sum_{c,kh,kw} weight[o,c,kh,kw] * x[b,c,h+kh-1,w+kw-1]
                             * exp(-8.3*|depth[b,h,w] - depth[b,h+kh-1,w+kw-1]|)

Sharding: core = 2*b + (h >= 128); each core computes a [32, 128, 256] output
slab from a 130-row padded input frame (1-row halo from the host slice).

v4 pipeline (bf16 datapath, DMA spread across SP + GpSimd + ACT queues,
triple-buffered loads, per-t sim granularity):
  A. sim: 3 row-view depth loads; per-t merged DVE sub+abs, per-t ACT exp,
     per-t DMA store -> DRAM simd[9, 32768] bf16.
  B. main loop over 8 tiles of 4096 px (16 rows):
     - 1 DMA (queue A): x chunk loaded 3x at flat offsets j-1 into the three
       partition groups of x3c [96, 18*258] bf16 (column shifts for free).
     - 3 DMAs (part t=0 on queue A, t=1,2 on queue B): simd rows {3t+j} tile
       window replicated to 32 partitions per j group -> simrep [96, 3*4096].
     - DVE: xm3[:, t] = x3c rows(t..t+16) * simrep(t)  (bf16 2x)  t=0,1,2
     - PE : psum[32@g, 4096] += w3[:, t].T @ xm3[:, t]  (K=96, N=512 x8, bf16)
     - ACT: psum -> out_sb bf16, then ACT-issued DMA out.
  Queue A = SP for even tiles, GpSimd for odd (B is the other one).
"""
import sys

import numpy as np

sys.path.insert(0, "/opt/trn_rl_repo")

import concourse.bass as bass
import concourse.mybir as mybir
from concourse.bass_utils import run_bass_kernel_spmd

F32 = mybir.dt.float32
BF16 = mybir.dt.bfloat16
EXP = mybir.ActivationFunctionType.Exp
COPY = mybir.ActivationFunctionType.Copy

B, C, H, W = 4, 32, 256, 256
O = 32
ALPHA = 8.3
R = 128  # output rows per core
WP = W + 2  # padded width
FR = R + 2  # frame rows per core
NPIX = R * W  # 32768
XLEN = FR * WP + 2  # flat x frame + 1-elem guard pads on both ends
TROWS = 16  # rows per tile
TILE = TROWS * W  # 4096
NT = R // TROWS  # 8
CH_ROWS = TROWS + 2  # x chunk rows
CH_FREE = CH_ROWS * WP  # 4644
MMN = 512  # matmul free-dim chunk (one PSUM bank)
QN = TILE // MMN  # 8
T3 = 3 * TILE
NBL = 3  # load-side buffers (x3c, simrep)


def build_nc():
    nc = bass.Bass("TRN2", target_bir_lowering=False, debug=False, num_devices=8)
    xb_in = nc.declare_dram_parameter("xb", [C, XLEN], BF16, isOutput=False)
    dp_in = nc.declare_dram_parameter("dp", [FR, WP], F32, isOutput=False)
    w3_in = nc.declare_dram_parameter("w3", [96, 96], BF16, isOutput=False)
    out_d = nc.declare_dram_parameter("outd", [O, NPIX], BF16, isOutput=True)
    simd = nc.dram_tensor("simd", [9, NPIX], BF16)

    from contextlib import ExitStack

    ctx = ExitStack()
    with ctx:
        d_sb = ctx.enter_context(nc.sbuf_tensor([128, 3 * WP], F32))
        adiff9 = ctx.enter_context(nc.sbuf_tensor([128, 9 * W], F32))
        sim9 = ctx.enter_context(nc.sbuf_tensor([128, 9 * W], BF16))
        w3_sb = ctx.enter_context(nc.sbuf_tensor([96, 96], BF16))
        x3c = ctx.enter_context(nc.sbuf_tensor([96, NBL * CH_FREE], BF16))
        simrep = ctx.enter_context(nc.sbuf_tensor([96, NBL * T3], BF16))
        xm3 = ctx.enter_context(nc.sbuf_tensor([96, 2 * T3], BF16))
        out_sb = ctx.enter_context(nc.sbuf_tensor([32, 2 * TILE], BF16))
        psum = ctx.enter_context(nc.psum_tensor([64, TILE], F32))
        ld_d = ctx.enter_context(nc.semaphore("ld_d"))
        ld_w = ctx.enter_context(nc.semaphore("ld_w"))
        x_e = ctx.enter_context(nc.semaphore("x_e"))
        x_o = ctx.enter_context(nc.semaphore("x_o"))
        sim_dve = ctx.enter_context(nc.semaphore("sim_dve"))
        act_exp = ctx.enter_context(nc.semaphore("act_exp"))
        sim_st = ctx.enter_context(nc.semaphore("sim_st"))
        b0_e = ctx.enter_context(nc.semaphore("b0_e"))
        b0_o = ctx.enter_context(nc.semaphore("b0_o"))
        b12_e = ctx.enter_context(nc.semaphore("b12_e"))
        b12_o = ctx.enter_context(nc.semaphore("b12_o"))
        mod_sem = ctx.enter_context(nc.semaphore("mod_sem"))
        pe_sem = ctx.enter_context(nc.semaphore("pe_sem"))
        act_cp = ctx.enter_context(nc.semaphore("act_cp"))
        st_e = ctx.enter_context(nc.semaphore("st_e"))
        st_o = ctx.enter_context(nc.semaphore("st_o"))
        block = ctx.enter_context(nc.Block())

        def rap(base_ap, offset, dims):
            return bass.AP(tensor=base_ap.tensor, offset=offset, ap=dims)

        def x3c_load(eng, i):
            # one DMA: 3 column shifts x 32 channels on partitions
            bl = i % NBL
            sem = x_e if i % 2 == 0 else x_o
            if i >= 2:
                # self-wait: prior completions of this sem have landed
                eng.wait_ge(sem, 16 * (i // 2))
            src = rap(
                xb_in.ap(),
                16 * i * WP,
                [[1, 3], [XLEN, C], [1, CH_FREE]],
            )
            eng.dma_start(
                x3c[:, bl * CH_FREE : (bl + 1) * CH_FREE], src
            ).then_inc(sem, 16)

        def brc_load(eng, i, t):
            # simd rows {3t, 3t+1, 3t+2} tile window replicated to 32
            # partitions per j group
            bl = i % NBL
            if t == 0:
                sem = b0_e if i % 2 == 0 else b0_o
                if i >= 2:
                    eng.wait_ge(sem, 16 * (i // 2))
            else:
                sem = b12_e if i % 2 == 0 else b12_o
                if i >= 2 and t == 1:
                    eng.wait_ge(sem, 32 * (i // 2))
            src = rap(
                simd.ap(),
                3 * t * NPIX + i * TILE,
                [[NPIX, 3], [0, C], [1, TILE]],
            )
            eng.dma_start(
                simrep[:, bl * T3 + t * TILE : bl * T3 + (t + 1) * TILE],
                src,
            ).then_inc(sem, 16)

        def reuse_wait(eng, i):
            # x3c/simrep buffer i%NBL was consumed by tile i-NBL's 3 muls
            if i >= NBL:
                eng.wait_ge(mod_sem, 3 * (i - NBL) + 3)

        @block.sync
        def _(sync: bass.BassEngine):
            # depth views: order 1,0,2 (sub_t=0 needs views 0 and 1)
            for t in (1, 0, 2):
                sync.dma_start(
                    d_sb[:, t * WP : (t + 1) * WP], dp_in[t : t + 128, :]
                ).then_inc(ld_d, 16)
            sync.dma_start(w3_sb[:], w3_in[:]).then_inc(ld_w, 16)
            x3c_load(sync, 0)
            # sim -> DRAM per t part [128 rows, 3 taps, 256]
            for t in range(3):
                sync.wait_ge(act_exp, t + 1)
                sync.dma_start(
                    rap(
                        simd.ap(),
                        3 * t * NPIX,
                        [[W, 128], [NPIX, 3], [1, W]],
                    ),
                    sim9[:, 3 * t * W : 3 * (t + 1) * W].rearrange(
                        "p (k w) -> p k w", k=3
                    ),
                ).then_inc(sim_st, 16)
            sync.wait_ge(sim_st, 48)
            brc_load(sync, 0, 0)
            brc_load(sync, 1, 1)
            brc_load(sync, 1, 2)
            for i in range(2, NT):
                reuse_wait(sync, i)
                if i % 2 == 0:
                    x3c_load(sync, i)
                    brc_load(sync, i, 0)
                else:
                    brc_load(sync, i, 1)
                    brc_load(sync, i, 2)

        @block.gpsimd
        def _(pool):
            x3c_load(pool, 1)
            pool.wait_ge(sim_st, 48)
            brc_load(pool, 0, 1)
            brc_load(pool, 0, 2)
            brc_load(pool, 1, 0)
            for i in range(2, NT):
                reuse_wait(pool, i)
                if i % 2 == 0:
                    brc_load(pool, i, 1)
                    brc_load(pool, i, 2)
                else:
                    x3c_load(pool, i)
                    brc_load(pool, i, 0)

        @block.vector
        def _(vector):
            # sim phase: per-t merged diff + abs over 3 taps
            d_ap = d_sb[:, 0:1]
            ad_ap = adiff9[:, 0:1]
            for t in range(3):
                vector.wait_ge(ld_d, 48)
                center = rap(d_ap, WP + 1, [[3 * WP, 128], [0, 3], [1, W]])
                wins = rap(d_ap, t * WP, [[3 * WP, 128], [1, 3], [1, W]])
                av = rap(ad_ap, 3 * t * W, [[9 * W, 128], [W, 3], [1, W]])
                vector.tensor_sub(av, center, wins)
                vector.drain()
                vector.scalar_tensor_tensor(
                    av,
                    av,
                    -1.0,
                    av,
                    op0=mybir.AluOpType.mult,
                    op1=mybir.AluOpType.max,
                ).then_inc(sim_dve, 1)
            # modulation loop
            for i in range(NT):
                bl = i % NBL
                sb = i % 2
                half = i // 2 + 1
                vector.wait_ge(x_e if sb == 0 else x_o, 16 * half)
                if i >= 2:
                    vector.wait_ge(pe_sem, i - 1)
                xv = x3c[:, bl * CH_FREE : (bl + 1) * CH_FREE].rearrange(
                    "p (r w) -> p r w", w=WP
                )
                for t in range(3):
                    if t == 0:
                        vector.wait_ge(b0_e if sb == 0 else b0_o, 16 * half)
                    elif t == 1:
                        vector.wait_ge(b12_e if sb == 0 else b12_o, 32 * half)
                    sv = simrep[
                        :, bl * T3 + t * TILE : bl * T3 + (t + 1) * TILE
                    ].rearrange("p (r w) -> p r w", w=W)
                    vector.tensor_mul(
                        xm3[
                            :, sb * T3 + t * TILE : sb * T3 + (t + 1) * TILE
                        ].rearrange("p (r w) -> p r w", w=W),
                        xv[:, t : t + TROWS, 1 : 1 + W],
                        sv,
                    ).then_inc(mod_sem, 1)

        @block.tensor
        def _(tensor):
            tensor.wait_ge(ld_w, 16)
            for i in range(NT):
                sb = i % 2
                g = i % 2
                if i >= 2:
                    tensor.wait_ge(act_cp, i - 1)
                for t in range(3):
                    tensor.wait_ge(mod_sem, 3 * i + t + 1)
                    for q in range(QN):
                        mm = tensor.matmul(
                            psum[32 * g : 32 * (g + 1), q * MMN : (q + 1) * MMN],
                            w3_sb[:, 32 * t : 32 * (t + 1)],
                            xm3[
                                :,
                                sb * T3
                                + t * TILE
                                + q * MMN : sb * T3
                                + t * TILE
                                + (q + 1) * MMN,
                            ],
                            start=(t == 0),
                            stop=(t == 2),
                        )
                        if t == 2 and q == QN - 1:
                            mm.then_inc(pe_sem, 1)

        @block.scalar
        def _(scalar):
            # per-t exp over 3 taps (bf16 out)
            for t in range(3):
                scalar.wait_ge(sim_dve, t + 1)
                scalar.activation(
                    sim9[:, 3 * t * W : 3 * (t + 1) * W],
                    adiff9[:, 3 * t * W : 3 * (t + 1) * W],
                    EXP,
                    scale=-ALPHA,
                ).then_inc(act_exp, 1)
            # psum -> sbuf copies (f32 -> bf16) + ACT-issued output stores
            for i in range(NT):
                sb = i % 2
                g = i % 2
                scalar.wait_ge(pe_sem, i + 1)
                if i >= 2:
                    scalar.wait_ge(st_e if sb == 0 else st_o, 16 * (i // 2))
                scalar.activation(
                    out_sb[:, sb * TILE : (sb + 1) * TILE],
                    psum[32 * g : 32 * (g + 1), :],
                    COPY,
                ).then_inc(act_cp, 1)
                scalar.wait_ge(act_cp, i + 1)
                scalar.dma_start(
                    out_d[:, i * TILE : (i + 1) * TILE],
                    out_sb[:, sb * TILE : (sb + 1) * TILE],
                ).then_inc(st_e if sb == 0 else st_o, 16)

    return nc


_NC_CACHE = None


def _get_nc():
    global _NC_CACHE
    if _NC_CACHE is None:
        _NC_CACHE = build_nc()
    return _NC_CACHE


def _prep_core(x_bf, depth, core):
    import ml_dtypes

    b, half = core // 2, core % 2
    r0 = half * R
    # padded frame rows r0-1 .. r0+R (inclusive), zero-padded cols
    xpad = np.zeros((C, FR, WP), dtype=ml_dtypes.bfloat16)
    dpad = np.zeros((FR, WP), dtype=np.float32)
    lo, hi = r0 - 1, r0 + R + 1
    slo, shi = max(lo, 0), min(hi, H)
    xpad[:, slo - lo : shi - lo, 1 : 1 + W] = x_bf[b, :, slo:shi, :]
    dpad[slo - lo : shi - lo, 1 : 1 + W] = depth[b, 0, slo:shi, :]
    xb = np.zeros((C, XLEN), dtype=ml_dtypes.bfloat16)
    xb[:, 1 : 1 + FR * WP] = xpad.reshape(C, FR * WP)
    return {"xb": xb, "dp": dpad, "w3": None}


def kernel(x, depth, weight):
    import ml_dtypes

    x = np.ascontiguousarray(x, dtype=np.float32)
    depth = np.ascontiguousarray(depth, dtype=np.float32)
    weight = np.ascontiguousarray(weight, dtype=np.float32)

    x_bf = x.astype(ml_dtypes.bfloat16)
    # w3[32j + c, 32t + o] = weight[o, c, t, j]
    w3 = (
        np.transpose(weight, (3, 1, 2, 0))
        .reshape(96, 96)
        .astype(ml_dtypes.bfloat16)
    )

    in_maps = []
    for core in range(8):
        m = _prep_core(x_bf, depth, core)
        m["w3"] = w3
        in_maps.append(m)

    nc = _get_nc()
    res = run_bass_kernel_spmd(nc, in_maps, list(range(8)))

    out = np.empty((B, O, H, W), dtype=np.float32)
    for core in range(8):
        b, half = core // 2, core % 2
        out[b, :, half * R : (half + 1) * R, :] = (
            res.results[core]["outd"].astype(np.float32).reshape(O, R, W)
        )
    return out
